# revision 23
# baseline (speedup 1.0000x reference)
"""Trainium2 Bass kernel for 16-head causal MHA (B=4, S=2048, E=1024, D=64).

Sharding: 8 cores = 4 batches x 2 head-halves; core c handles batch c//2,
head-half c%2. All wire traffic is bf16 and deduplicated with on-device
collectives so each unique byte crosses the host<->device link once:

  - x: core c ships 1/8 of x (1024 rows); a pair AllGather {2b, 2b+1}
    reconstructs the full x[b] on device.
  - Wq/Wk/Wv: core c ships one head-pair slab; an AllGather over
    {0,2,4,6} / {1,3,5,7} gives each core its half's 4 pairs, at local
    pair indices (the program is identical on every core; all per-core
    differences are in the input data).
  - Wo: transposed + column-halved per head-half group, sharded 2 pairs
    per core, 8-way AllGather -> global-pair-indexed [8,128,512] tiles.
  - ctx: after each pair's attention, a pair AllGather shares its ctx so
    each core computes its own 512 output columns of out[b] (the output
    projection needs all 16 heads).

Each core returns out[b][:, hh*512:(hh+1)*512] in bf16 (pre-bias); the
host interleaves column halves and adds bo_eff = bo + bv @ Wo.T (softmax
rows sum to 1, so the V-bias routes through Wo as a constant vector).

Matmul inputs are bf16 (full PE rate), accumulation in f32 PSUM.
"""
import numpy as np

B, S, E = 4, 2048, 1024
H, D = 16, 64
NP = 4     # head-pairs per core (2 heads packed per 128-wide tile)
KT = 8     # E / 128 contraction tiles
NQB = 4    # q blocks of 512
NTT = 16   # t tiles of 128

_NC = None
_RUNNER = None
_SIM_NO_CC = False  # probe: replace collectives with local DMAs (timing only)


def _build():
    import concourse.bacc as bacc
    import concourse.tile as tile
    from concourse import mybir
    from concourse.masks import make_identity, make_upper_triangular

    f32, f32r = mybir.dt.float32, mybir.dt.float32r
    bf16 = mybir.dt.bfloat16
    Act = mybir.ActivationFunctionType

    nc = bacc.Bacc("TRN2", num_devices=8)
    XS = nc.dram_tensor("xs", [1024, E], bf16, kind="ExternalInput")
    WQKVS = nc.dram_tensor("wqkvs", [3, KT, 128, 128], bf16, kind="ExternalInput")
    WOS = nc.dram_tensor("wos", [2, 128, 512], bf16, kind="ExternalInput")
    BQK = nc.dram_tensor("bqk", [2, NP, 128, 1], f32, kind="ExternalInput")
    OUT = nc.dram_tensor("out", [S, 512], bf16, kind="ExternalOutput")

    PAIR_GROUPS = [[0, 1], [2, 3], [4, 5], [6, 7]]
    HALF_GROUPS = [[0, 2, 4, 6], [1, 3, 5, 7]]

    def allgather(groups, in_tile, out_tile):
        if _SIM_NO_CC:
            n = len(groups[0])
            chunk = out_tile.shape[0] // n
            for r in range(n):
                dst = (out_tile[r] if chunk == 1
                       else out_tile[r * chunk:(r + 1) * chunk])
                nc.sync.dma_start(dst, in_tile)
            return
        nc.gpsimd.collective_compute(
            "AllGather", mybir.AluOpType.bypass,
            replica_groups=groups, ins=[in_tile.opt()], outs=[out_tile.opt()])

    with tile.TileContext(nc) as tc:
        with tc.tile_pool(name="dram", bufs=1, space="DRAM") as dp, \
             tc.tile_pool(name="persist", bufs=1) as pers:
            # ---- DRAM scratch: collective bounces + gather outputs ----
            xsb = dp.tile([1024, E], bf16, name="xsb")
            xg = dp.tile([2, 1024, E], bf16, name="xg")
            wqkvb = dp.tile([3, KT, 128, 128], bf16, name="wqkvb")
            wqkvg = dp.tile([NP, 3, KT, 128, 128], bf16, name="wqkvg")
            wob = dp.tile([2, 128, 512], bf16, name="wob")
            wog = dp.tile([8, 128, 512], bf16, name="wog")
            cin = [dp.tile([128, S], bf16, name=f"cin{p}") for p in range(NP)]
            cg = [dp.tile([2, 128, S], bf16, name=f"cg{p}") for p in range(NP)]

            # bounce + input gathers; x first (everything depends on it)
            nc.sync.dma_start(xsb, XS.ap())
            allgather(PAIR_GROUPS, xsb, xg)
            nc.sync.dma_start(wqkvb, WQKVS.ap())
            allgather(HALF_GROUPS, wqkvb, wqkvg)
            nc.sync.dma_start(wob, WOS.ap())
            allgather(HALF_GROUPS, wob, wog)

            # ---- persistent SBUF constants ----
            ident_f = pers.tile([128, 128], f32)
            make_identity(nc, ident_f)
            ident_b = pers.tile([128, 128], bf16)
            nc.vector.tensor_copy(ident_b, ident_f)
            ones16 = pers.tile([128, NTT, 1], bf16)
            nc.vector.memset(ones16, 1.0)
            onesrow_f = pers.tile([1, 64], f32)
            nc.vector.memset(onesrow_f, 1.0)
            ones_row = pers.tile([1, 64], f32r)
            nc.vector.tensor_copy(ones_row, onesrow_f)
            tri_f = pers.tile([128, 128], f32)
            make_upper_triangular(nc, tri_f, val=1.0, diag=True)
            tri_b = pers.tile([128, 128], bf16)
            nc.vector.tensor_copy(tri_b, tri_f)
            zeros_b = pers.tile([128, 384], bf16)
            nc.vector.memset(zeros_b, 0.0)

            bias_t = pers.tile([128, 2, NP, 1], f32, name="bias_t")
            nc.sync.dma_start(bias_t, BQK.ap().rearrange("t p i one -> i t p one"))
            bq_t = [bias_t[:, 0, p] for p in range(NP)]
            bk_t = [bias_t[:, 1, p] for p in range(NP)]

            with tc.tile_pool(name="xtp", bufs=1) as xtp:
                xT = [xtp.tile([128, S], bf16, name=f"xT{i}") for i in range(KT)]

                # ---- Phase A: x -> x^T (bf16) via PE transpose ----
                with tc.tile_pool(name="stA", bufs=2) as sa, \
                     tc.tile_pool(name="psA", bufs=4, space="PSUM") as pA:
                    for k in range(KT):
                        colblk = sa.tile([128, NTT, 128], bf16)
                        for h2 in range(2):
                            src = xg[h2].rearrange("(st p) e -> p st e", p=128)
                            nc.sync.dma_start(
                                colblk[:, h2 * 8:(h2 + 1) * 8, :],
                                src[:, :, k * 128:(k + 1) * 128])
                        for st in range(NTT):
                            tp = pA.tile([128, 128], bf16)
                            nc.tensor.transpose(tp, colblk[:, st, :], ident_b)
                            nc.vector.tensor_copy(xT[k][:, st * 128:(st + 1) * 128], tp)

                # ---- Phases B+C: per pair, QKV projection then attention ----
                with tc.tile_pool(name="qtp", bufs=2) as qtp, \
                     tc.tile_pool(name="ktp", bufs=2) as ktp, \
                     tc.tile_pool(name="vnp", bufs=2) as vnp, \
                     tc.tile_pool(name="stB", bufs=3) as sb_, \
                     tc.tile_pool(name="vt2", bufs=1) as vt2p, \
                     tc.tile_pool(name="expp", bufs=6) as expp, \
                     tc.tile_pool(name="rp", bufs=4) as rp, \
                     tc.tile_pool(name="ctxp", bufs=2) as ctxp, \
                     tc.tile_pool(name="psB", bufs=4, space="PSUM") as pB, \
                     tc.tile_pool(name="psCTX", bufs=1, space="PSUM") as psCTX:
                    for p in range(NP):
                        qt = qtp.tile([128, S], bf16, name="qt")
                        kt = ktp.tile([128, S], bf16, name="kt")
                        vn = vnp.tile([128, 2, NTT, 65], bf16, name="vn")
                        vt2 = vt2p.tile([128, S], bf16)
                        ctxn = ctxp.tile([128, S], bf16, name="ctxn")

                        # QKV projections (transposed, 2-head packed)
                        for wi, (bias_, dest) in enumerate(
                                ((bq_t[p], qt), (bk_t[p], kt), (None, vt2))):
                            wf = sb_.tile([128, KT, 128], bf16, name="wf", bufs=4)
                            nc.sync.dma_start(
                                wf, wqkvg[p, wi].rearrange("k i j -> i k j"))
                            wrs = [wf[:, k, :] for k in range(KT)]
                            for half in range(2):
                                pss = [pB.tile([128, 512], f32, name="pss", bufs=2)
                                       for _ in range(2)]
                                for k in range(KT):
                                    for i in range(2):
                                        nb = 2 * half + i
                                        nc.tensor.matmul(
                                            pss[i], wrs[k],
                                            xT[k][:, nb * 512:(nb + 1) * 512],
                                            start=(k == 0), stop=(k == KT - 1),
                                        )
                                for i in range(2):
                                    nb = 2 * half + i
                                    dslc = dest[:, nb * 512:(nb + 1) * 512]
                                    if bias_ is not None:
                                        nc.vector.tensor_scalar_add(dslc, pss[i], bias_)
                                    else:
                                        nc.vector.tensor_copy(dslc, pss[i])
                        # V back to natural [t, d] layout, split per head + ones col
                        for h in range(2):
                            nc.vector.tensor_copy(vn[:, h, :, 64:65], ones16)
                        for tt in range(NTT):
                            tp2 = pB.tile([128, 128], bf16, name="sc", bufs=4)
                            nc.tensor.transpose(tp2, vt2[:, tt * 128:(tt + 1) * 128], ident_b)
                            for h in range(2):
                                nc.vector.tensor_copy(
                                    vn[:, h, tt, 0:64], tp2[:, h * 64:(h + 1) * 64])

                        # attention for this pair
                        for qb in range(NQB):
                            T = 4 * (qb + 1)  # causal: t-tiles 0..T-1
                            cps = [psCTX.tile([65, 512], f32, name=f"cps{h}")
                                   for h in range(2)]
                            prev_exp = None
                            for tt in range(T):
                                scs = []
                                for h in range(2):
                                    sc = pB.tile([128, 512], f32, name="sc", bufs=4)
                                    nc.tensor.matmul(
                                        sc,
                                        kt[h * 64:(h + 1) * 64, tt * 128:(tt + 1) * 128],
                                        qt[h * 64:(h + 1) * 64, qb * 512:(qb + 1) * 512],
                                        start=True, stop=True,
                                    )
                                    scs.append(sc)
                                if prev_exp is not None:
                                    for h in range(2):
                                        nc.tensor.matmul(
                                            cps[h], vn[:, h, tt - 1, :], prev_exp[h],
                                            start=(tt - 1 == 0), stop=False,
                                        )
                                j = tt - 4 * qb  # >=0 on diagonal tiles
                                cur = []
                                for h in range(2):
                                    ex = expp.tile([128, 512], bf16)
                                    if j >= 1:
                                        nc.gpsimd.tensor_copy(
                                            ex[:, 0:j * 128], zeros_b[:, 0:j * 128])
                                    if j >= 0:
                                        nc.scalar.activation(
                                            ex[:, j * 128:512], scs[h][:, j * 128:512],
                                            Act.Exp, scale=0.125)
                                        nc.vector.tensor_mul(
                                            ex[:, j * 128:(j + 1) * 128],
                                            ex[:, j * 128:(j + 1) * 128], tri_b)
                                    else:
                                        nc.scalar.activation(ex, scs[h], Act.Exp, scale=0.125)
                                    cur.append(ex)
                                prev_exp = cur
                            for h in range(2):
                                nc.tensor.matmul(
                                    cps[h], vn[:, h, T - 1, :], prev_exp[h],
                                    start=(T - 1 == 0), stop=True,
                                )
                            # evict cps to SBUF fast (frees PSUM banks), then
                            # denominators (row 64) -> bcast -> reciprocal -> normalize
                            for h in range(2):
                                csb = rp.tile([65, 512], f32, name="csb", bufs=3)
                                nc.scalar.copy(csb, cps[h])
                                rh = rp.tile([1, 512], f32r, name="rh")
                                nc.vector.tensor_copy(rh, csb[64:65, :])
                                rb = pB.tile([64, 512], f32, name="sc", bufs=4)
                                nc.tensor.matmul(rb, ones_row, rh, start=True, stop=True)
                                rbs = rp.tile([64, 512], f32, name="rbs")
                                nc.vector.reciprocal(rbs, rb)
                                nc.vector.tensor_mul(
                                    ctxn[h * 64:(h + 1) * 64, qb * 512:(qb + 1) * 512],
                                    csb[0:64, :], rbs,
                                )
                        # share this pair's ctx with the peer core
                        nc.sync.dma_start(cin[p], ctxn)
                        allgather(PAIR_GROUPS, cin[p], cg[p])

                # ---- Phase D: output projection, my 512 columns, all 16 heads ----
                with tc.tile_pool(name="stD", bufs=3) as sd, \
                     tc.tile_pool(name="wo2", bufs=1) as wop, \
                     tc.tile_pool(name="ctxg", bufs=1) as cgp, \
                     tc.tile_pool(name="psD", bufs=4, space="PSUM") as pD:
                    wo_sb, ctx_sb = [], []
                    for gp in range(8):
                        g, p = gp // NP, gp % NP
                        w2 = wop.tile([128, 512], bf16, name=f"wo2_{gp}")
                        nc.sync.dma_start(w2, wog[gp])
                        wo_sb.append(w2)
                        c2 = cgp.tile([128, S], bf16, name=f"cg2_{gp}")
                        nc.sync.dma_start(c2, cg[p][g])
                        ctx_sb.append(c2)
                    for qt_i in range(NTT):
                        ob = sd.tile([128, 512], bf16, name="ob")
                        ps = pD.tile([128, 512], f32, name="psd")
                        for gp in range(8):
                            nc.tensor.matmul(
                                ps,
                                ctx_sb[gp][:, qt_i * 128:(qt_i + 1) * 128],
                                wo_sb[gp],
                                start=(gp == 0), stop=(gp == 7),
                            )
                        nc.vector.tensor_copy(ob, ps)
                        nc.sync.dma_start(OUT.ap()[qt_i * 128:(qt_i + 1) * 128, :], ob)

    nc.finalize()
    return nc


def _get_nc():
    global _NC
    if _NC is None:
        _NC = _build()
    return _NC


def _pack_w_into(dst, Wh_bf):
    # dst: contiguous [NP, KT, 128, 128] bf16 view;
    # dst[p,k,i,j] = Wh[2p + j//64, k*128+i, j%64] via one fused strided copy
    dv = dst.reshape(NP, KT, 128, 2, D)
    dv[...] = Wh_bf.reshape(NP, 2, KT, 128, D).transpose(0, 2, 3, 1, 4)


def _global_xs(x):
    import ml_dtypes
    return np.asarray(x, np.float32).reshape(8 * 1024, E).astype(ml_dtypes.bfloat16)


def _global_rest(Wq, bq, Wk, bk, Wv, Wo):
    """Concatenated-over-cores weight/bias arrays (shardable on axis 0)."""
    import ml_dtypes
    bf = ml_dtypes.bfloat16
    Wo = np.asarray(Wo, np.float32)

    # core c = 2b + hh ships its half's pair b of each of q/k/v
    wqkvs = np.empty((NP, 2, 3, KT, 128, 128), bf)
    for wi, W in enumerate((Wq, Wk, Wv)):
        Wb = np.asarray(W, np.float32).astype(bf)
        for hh in range(2):
            _pack_w_into(wqkvs[:, hh, wi], Wb[hh * 8:hh * 8 + 8])
    wqkvs = wqkvs.reshape(8 * 3, KT, 128, 128)

    # Wo transposed, grouped by global head-pair: woT[gp, i, e] = Wo[e, 128*gp + i]
    woT = np.ascontiguousarray(Wo.T).astype(bf).reshape(8, 128, E)
    wos = np.empty((NP, 2, 2, 128, 512), bf)
    for hh in range(2):
        wos[:, hh] = woT[:, :, hh * 512:(hh + 1) * 512].reshape(NP, 2, 128, 512)
    wos = wos.reshape(16, 128, 512)

    bq = np.asarray(bq, np.float32)
    bk = np.asarray(bk, np.float32)
    bqk = np.empty((NP, 2, 2, NP, 128, 1), np.float32)
    for hh in range(2):
        bqk[:, hh, 0] = bq[hh * 8:hh * 8 + 8].reshape(NP, 128, 1)[None]
        bqk[:, hh, 1] = bk[hh * 8:hh * 8 + 8].reshape(NP, 128, 1)[None]
    bqk = bqk.reshape(16, NP, 128, 1)
    return {"wqkvs": wqkvs, "wos": wos, "bqk": bqk}


def build_inputs(x, Wq, bq, Wk, bk, Wv, bv, Wo, bo):
    """Per-core input dicts (list of 8) for the SPMD kernel."""
    xb = _global_xs(x)
    rest = _global_rest(Wq, bq, Wk, bk, Wv, Wo)
    in_maps = []
    for c in range(8):
        in_maps.append({
            "xs": xb[c * 1024:(c + 1) * 1024],
            "wqkvs": rest["wqkvs"][3 * c:3 * c + 3],
            "wos": rest["wos"][2 * c:2 * c + 2],
            "bqk": rest["bqk"][2 * c:2 * c + 2],
        })
    return in_maps


def assemble_output(parts, Wo, bo, bv):
    """parts: [8, S, 512] bf16 -> full [B, S, E] f32 with bias."""
    Wo = np.asarray(Wo, np.float32)
    bo_eff = (np.asarray(bo, np.float32)
              + np.asarray(bv, np.float32).reshape(-1) @ Wo.T)
    out = np.empty((B, S, E), np.float32)
    for c in range(8):
        b, hh = divmod(c, 2)
        cols = slice(hh * 512, (hh + 1) * 512)
        np.add(parts[c], bo_eff[cols], out=out[b, :, cols])
    return out


def _make_runner(nc):
    """Cached jitted shard_map runner over 8 cores (axon / PJRT path).

    Output zero-buffers and the partition-id tensor are created on device
    inside the jitted function, so per call only the real inputs cross the
    host->device link.
    """
    import jax
    import jax.numpy as jnp
    from jax.sharding import Mesh, PartitionSpec, NamedSharding
    try:
        from jax.experimental.shard_map import shard_map
    except ImportError:
        from jax.shard_map import shard_map
    from concourse import bass2jax, mybir
    from concourse.bass2jax import _bass_exec_p, install_neuronx_cc_hook

    install_neuronx_cc_hook()

    partition_name = nc.partition_id_tensor.name if nc.partition_id_tensor else None
    dbg_name = nc.dbg_addr.name if nc.dbg_addr is not None else None

    in_names, out_names, out_avals = [], [], []
    for alloc in nc.m.functions[0].allocations:
        if not isinstance(alloc, mybir.MemoryLocationSet):
            continue
        name = alloc.memorylocations[0].name
        if alloc.kind == "ExternalInput":
            if name not in (partition_name, dbg_name):
                in_names.append(name)
        elif alloc.kind == "ExternalOutput":
            out_names.append(name)
            out_avals.append(jax.core.ShapedArray(
                tuple(alloc.tensor_shape), mybir.dt.np(alloc.dtype)))

    bind_names = list(in_names)
    if dbg_name is not None:
        bind_names.append(dbg_name)
    bind_names.extend(out_names)
    if partition_name is not None:
        bind_names.append(partition_name)

    def _body(*args):
        operands = list(args)
        if partition_name is not None:
            operands.append(bass2jax.partition_id_tensor())
        outs = _bass_exec_p.bind(
            *operands,
            out_avals=tuple(out_avals),
            in_names=tuple(bind_names),
            out_names=tuple(out_names),
            lowering_input_output_aliases=(),
            sim_require_finite=True,
            sim_require_nnan=True,
            nc=nc,
        )
        return tuple(outs)

    devices = jax.devices()[:8]
    mesh = Mesh(np.asarray(devices), ("core",))
    sharding = NamedSharding(mesh, PartitionSpec("core"))
    n_extra = (1 if dbg_name is not None else 0) + len(out_names)
    jitted = jax.jit(shard_map(
        _body, mesh=mesh,
        in_specs=(PartitionSpec("core"),) * (len(in_names) + n_extra),
        out_specs=(PartitionSpec("core"),) * len(out_names),
        check_rep=False,
    ))
    # device-resident zero buffers (output initializers + dbg), reused
    # across calls so they never cross the host->device link again
    extras = []
    if dbg_name is not None:
        extras.append(jax.device_put(np.zeros((8, 2), np.uint32), sharding))
    for a in out_avals:
        extras.append(jax.device_put(
            np.zeros((8 * a.shape[0], *a.shape[1:]), a.dtype), sharding))
    return jitted, in_names, out_names, out_avals, sharding, extras


def _get_runner():
    global _RUNNER
    if _RUNNER is None:
        _RUNNER = _make_runner(_get_nc())
    return _RUNNER


def kernel(x, Wq, bq, Wk, bk, Wv, bv, Wo, bo):
    try:
        import jax
        use_jax = sum(d.platform != "cpu" for d in jax.devices()) >= 8
    except Exception:
        use_jax = False

    if use_jax:
        import jax
        jitted, in_names, out_names, out_avals, sharding, extras = _get_runner()
        # start the big x transfer first; pack weights while it moves
        puts = {"xs": jax.device_put(_global_xs(x), sharding)}
        rest = _global_rest(Wq, bq, Wk, bk, Wv, Wo)
        for nm in ("wqkvs", "wos", "bqk"):
            puts[nm] = jax.device_put(rest[nm], sharding)
        outs = jitted(*(puts[nm] for nm in in_names), *extras)
        i = out_names.index("out")
        parts = np.asarray(outs[i]).reshape(8, *out_avals[i].shape)
    else:
        from concourse.bass_utils import run_bass_kernel_spmd
        in_maps = build_inputs(x, Wq, bq, Wk, bk, Wv, bv, Wo, bo)
        res = run_bass_kernel_spmd(_get_nc(), in_maps, core_ids=list(range(8)))
        parts = np.stack([res.results[c]["out"] for c in range(8)])

    return assemble_output(parts, Wo, bo, bv)


# revision 38
# speedup vs baseline: 1.5537x; 1.5537x over previous
"""Trainium2 Bass kernel for 16-head causal MHA (B=4, S=2048, E=1024, D=64).

Sharding: 8 cores = 4 batches x 2 head-halves; core c handles batch c//2,
head-half c%2. All wire traffic is bf16 and deduplicated with on-device
collectives so each unique byte crosses the host<->device link once:

  - x: core c ships 1/8 of x (1024 rows); a pair AllGather {2b, 2b+1}
    reconstructs the full x[b] on device.
  - Wq/Wk/Wv: core c ships one head-pair slab; an AllGather over
    {0,2,4,6} / {1,3,5,7} gives each core its half's 4 pairs, at local
    pair indices (the program is identical on every core; all per-core
    differences are in the input data).
  - Wo: transposed + column-halved per head-half group, sharded 2 pairs
    per core, 8-way AllGather -> global-pair-indexed [8,128,512] tiles.
  - ctx: after each pair's attention, a pair AllGather shares its ctx so
    each core computes its own 512 output columns of out[b] (the output
    projection needs all 16 heads).

Each core returns out[b][:, hh*512:(hh+1)*512] in bf16 (pre-bias); the
host interleaves column halves and adds bo_eff = bo + bv @ Wo.T (softmax
rows sum to 1, so the V-bias routes through Wo as a constant vector).

Matmul inputs are bf16 (full PE rate), accumulation in f32 PSUM.
"""
import numpy as np

B, S, E = 4, 2048, 1024
H, D = 16, 64
NP = 4     # head-pairs per core (2 heads packed per 128-wide tile)
KT = 8     # E / 128 contraction tiles
NQB = 4    # q blocks of 512
NTT = 16   # t tiles of 128

_NC = None
_RUNNER = None
_SIM_NO_CC = False  # probe: replace collectives with local DMAs (timing only)


def _build():
    import concourse.bacc as bacc
    import concourse.tile as tile
    from concourse import mybir
    from concourse.masks import make_identity, make_upper_triangular

    f32, f32r = mybir.dt.float32, mybir.dt.float32r
    bf16 = mybir.dt.bfloat16
    Act = mybir.ActivationFunctionType

    nc = bacc.Bacc("TRN2", num_devices=8)
    XS = nc.dram_tensor("xs", [1024, E], bf16, kind="ExternalInput")
    WQKVS = nc.dram_tensor("wqkvs", [3, KT, 128, 128], bf16, kind="ExternalInput")
    WOS = nc.dram_tensor("wos", [2, 128, 512], bf16, kind="ExternalInput")
    BQK = nc.dram_tensor("bqk", [2, NP, 128, 1], f32, kind="ExternalInput")
    OUT = nc.dram_tensor("out", [S, 512], bf16, kind="ExternalOutput")

    PAIR_GROUPS = [[0, 1], [2, 3], [4, 5], [6, 7]]
    HALF_GROUPS = [[0, 2, 4, 6], [1, 3, 5, 7]]

    def allgather(groups, in_tile, out_tile):
        if _SIM_NO_CC:
            n = len(groups[0])
            chunk = out_tile.shape[0] // n
            for r in range(n):
                dst = (out_tile[r] if chunk == 1
                       else out_tile[r * chunk:(r + 1) * chunk])
                nc.sync.dma_start(dst, in_tile)
            return
        nc.gpsimd.collective_compute(
            "AllGather", mybir.AluOpType.bypass,
            replica_groups=groups, ins=[in_tile.opt()], outs=[out_tile.opt()])

    with tile.TileContext(nc) as tc:
        with tc.tile_pool(name="dram", bufs=1, space="DRAM") as dp, \
             tc.tile_pool(name="persist", bufs=1) as pers:
            # ---- DRAM scratch: collective bounces + gather outputs ----
            xsb = dp.tile([1024, E], bf16, name="xsb")
            xg = dp.tile([2, 1024, E], bf16, name="xg")
            wqkvb = dp.tile([3, KT, 128, 128], bf16, name="wqkvb")
            wqkvg = dp.tile([NP, 3, KT, 128, 128], bf16, name="wqkvg")
            wob = dp.tile([2, 128, 512], bf16, name="wob")
            wog = dp.tile([8, 128, 512], bf16, name="wog")
            cin = [dp.tile([128, S], bf16, name=f"cin{p}") for p in range(NP)]
            cg = [dp.tile([2, 128, S], bf16, name=f"cg{p}") for p in range(NP)]

            # bounce + input gathers; x first (everything depends on it)
            nc.sync.dma_start(xsb, XS.ap())
            allgather(PAIR_GROUPS, xsb, xg)
            nc.sync.dma_start(wqkvb, WQKVS.ap())
            allgather(HALF_GROUPS, wqkvb, wqkvg)
            nc.sync.dma_start(wob, WOS.ap())
            allgather(HALF_GROUPS, wob, wog)

            # ---- persistent SBUF constants ----
            ident_f = pers.tile([128, 128], f32)
            make_identity(nc, ident_f)
            ident_b = pers.tile([128, 128], bf16)
            nc.vector.tensor_copy(ident_b, ident_f)
            ones16 = pers.tile([128, NTT, 1], bf16)
            nc.vector.memset(ones16, 1.0)
            tri_f = pers.tile([128, 128], f32)
            make_upper_triangular(nc, tri_f, val=1.0, diag=True)
            tri_b = pers.tile([128, 128], bf16)
            nc.vector.tensor_copy(tri_b, tri_f)
            zeros_b = pers.tile([128, 384], bf16)
            nc.vector.memset(zeros_b, 0.0)

            bias_t = pers.tile([128, 2, NP, 1], f32, name="bias_t")
            nc.sync.dma_start(bias_t, BQK.ap().rearrange("t p i one -> i t p one"))
            bq_t = [bias_t[:, 0, p] for p in range(NP)]
            bk_t = [bias_t[:, 1, p] for p in range(NP)]

            with tc.tile_pool(name="xtp", bufs=1) as xtp:
                xT = [xtp.tile([128, S], bf16, name=f"xT{i}") for i in range(KT)]

                # ---- Phase A: x -> x^T (bf16) via PE transpose ----
                with tc.tile_pool(name="stA", bufs=2) as sa, \
                     tc.tile_pool(name="psA", bufs=4, space="PSUM") as pA:
                    for k in range(KT):
                        colblk = sa.tile([128, NTT, 128], bf16)
                        for h2 in range(2):
                            src = xg[h2].rearrange("(st p) e -> p st e", p=128)
                            nc.sync.dma_start(
                                colblk[:, h2 * 8:(h2 + 1) * 8, :],
                                src[:, :, k * 128:(k + 1) * 128])
                        for st in range(NTT):
                            tp = pA.tile([128, 128], bf16)
                            nc.tensor.transpose(tp, colblk[:, st, :], ident_b)
                            nc.vector.tensor_copy(xT[k][:, st * 128:(st + 1) * 128], tp)

                # ---- Phases B+C: per pair, QKV projection then attention ----
                with tc.tile_pool(name="qtp", bufs=2) as qtp, \
                     tc.tile_pool(name="ktp", bufs=2) as ktp, \
                     tc.tile_pool(name="vnp", bufs=2) as vnp, \
                     tc.tile_pool(name="stB", bufs=3) as sb_, \
                     tc.tile_pool(name="vt2", bufs=1) as vt2p, \
                     tc.tile_pool(name="expp", bufs=6) as expp, \
                     tc.tile_pool(name="rp", bufs=4) as rp, \
                     tc.tile_pool(name="ctxp", bufs=2) as ctxp, \
                     tc.tile_pool(name="psB", bufs=4, space="PSUM") as pB, \
                     tc.tile_pool(name="psCTX", bufs=1, space="PSUM") as psCTX:
                    for p in range(NP):
                        qt = qtp.tile([128, S], bf16, name="qt")
                        kt = ktp.tile([128, S], bf16, name="kt")
                        vn = vnp.tile([128, 2, NTT, 65], bf16, name="vn")
                        vt2 = vt2p.tile([128, S], bf16)
                        ctxn = ctxp.tile([128, S], bf16, name="ctxn")

                        # QKV projections (transposed, 2-head packed)
                        for wi, (bias_, dest) in enumerate(
                                ((bq_t[p], qt), (bk_t[p], kt), (None, vt2))):
                            wf = sb_.tile([128, KT, 128], bf16, name="wf", bufs=4)
                            wsrc = wqkvg[p, wi].rearrange("k i j -> i k j")
                            nc.sync.dma_start(wf[:, 0:KT // 2], wsrc[:, 0:KT // 2])
                            nc.sync.dma_start(wf[:, KT // 2:], wsrc[:, KT // 2:])
                            wrs = [wf[:, k, :] for k in range(KT)]
                            for half in range(2):
                                pss = [pB.tile([128, 512], f32, name="pss", bufs=2)
                                       for _ in range(2)]
                                for k in range(KT):
                                    for i in range(2):
                                        nb = 2 * half + i
                                        nc.tensor.matmul(
                                            pss[i], wrs[k],
                                            xT[k][:, nb * 512:(nb + 1) * 512],
                                            start=(k == 0), stop=(k == KT - 1),
                                        )
                                for i in range(2):
                                    nb = 2 * half + i
                                    dslc = dest[:, nb * 512:(nb + 1) * 512]
                                    if bias_ is not None:
                                        nc.vector.tensor_scalar_add(dslc, pss[i], bias_)
                                    else:
                                        nc.vector.tensor_copy(dslc, pss[i])
                        # V back to natural [t, d] layout, split per head + ones col
                        for h in range(2):
                            nc.vector.tensor_copy(vn[:, h, :, 64:65], ones16)
                        for tt in range(NTT):
                            tp2 = pB.tile([128, 128], bf16, name="sc", bufs=4)
                            nc.tensor.transpose(tp2, vt2[:, tt * 128:(tt + 1) * 128], ident_b)
                            for h in range(2):
                                nc.vector.tensor_copy(
                                    vn[:, h, tt, 0:64], tp2[:, h * 64:(h + 1) * 64])

                        # attention for this pair
                        for qb in range(NQB):
                            T = 4 * (qb + 1)  # causal: t-tiles 0..T-1
                            cps = [psCTX.tile([65, 512], f32, name=f"cps{h}")
                                   for h in range(2)]
                            prev_exp = None
                            for tt in range(T):
                                scs = []
                                for h in range(2):
                                    sc = pB.tile([128, 512], f32, name="sc", bufs=4)
                                    nc.tensor.matmul(
                                        sc,
                                        kt[h * 64:(h + 1) * 64, tt * 128:(tt + 1) * 128],
                                        qt[h * 64:(h + 1) * 64, qb * 512:(qb + 1) * 512],
                                        start=True, stop=True,
                                    )
                                    scs.append(sc)
                                if prev_exp is not None:
                                    for h in range(2):
                                        nc.tensor.matmul(
                                            cps[h], vn[:, h, tt - 1, :], prev_exp[h],
                                            start=(tt - 1 == 0), stop=False,
                                        )
                                j = tt - 4 * qb  # >=0 on diagonal tiles
                                cur = []
                                for h in range(2):
                                    ex = expp.tile([128, 512], bf16)
                                    if j >= 1:
                                        nc.gpsimd.tensor_copy(
                                            ex[:, 0:j * 128], zeros_b[:, 0:j * 128])
                                    if j >= 0:
                                        nc.scalar.activation(
                                            ex[:, j * 128:512], scs[h][:, j * 128:512],
                                            Act.Exp, scale=0.125)
                                        nc.vector.tensor_mul(
                                            ex[:, j * 128:(j + 1) * 128],
                                            ex[:, j * 128:(j + 1) * 128], tri_b)
                                    else:
                                        nc.scalar.activation(ex, scs[h], Act.Exp, scale=0.125)
                                    cur.append(ex)
                                prev_exp = cur
                            for h in range(2):
                                nc.tensor.matmul(
                                    cps[h], vn[:, h, T - 1, :], prev_exp[h],
                                    start=(T - 1 == 0), stop=True,
                                )
                            # evict cps to SBUF fast (frees PSUM banks), then
                            # reciprocal of denominators (row 64) -> partition
                            # broadcast (gpsimd) -> normalize; no PE involved,
                            # so the PE queue never waits on this chain
                            for h in range(2):
                                csb = rp.tile([65, 512], f32, name="csb", bufs=3)
                                nc.scalar.copy(csb, cps[h])
                                rr = rp.tile([1, 512], f32, name="rr")
                                nc.vector.reciprocal(rr, csb[64:65, :])
                                rbs = rp.tile([64, 512], f32, name="rbs")
                                nc.gpsimd.partition_broadcast(rbs, rr)
                                nc.vector.tensor_mul(
                                    ctxn[h * 64:(h + 1) * 64, qb * 512:(qb + 1) * 512],
                                    csb[0:64, :], rbs,
                                )
                        # share this pair's ctx with the peer core
                        nc.sync.dma_start(cin[p], ctxn)
                        allgather(PAIR_GROUPS, cin[p], cg[p])

                # ---- Phase D: output projection, my 512 columns, all 16 heads ----
                with tc.tile_pool(name="stD", bufs=3) as sd, \
                     tc.tile_pool(name="wo2", bufs=1) as wop, \
                     tc.tile_pool(name="ctxg", bufs=1) as cgp, \
                     tc.tile_pool(name="psD", bufs=4, space="PSUM") as pD:
                    wo_sb, ctx_sb = [], []
                    for gp in range(8):
                        g, p = gp // NP, gp % NP
                        w2 = wop.tile([128, 512], bf16, name=f"wo2_{gp}")
                        nc.sync.dma_start(w2, wog[gp])
                        wo_sb.append(w2)
                        c2 = cgp.tile([128, S], bf16, name=f"cg2_{gp}")
                        nc.sync.dma_start(c2, cg[p][g])
                        ctx_sb.append(c2)
                    for qt_i in range(NTT):
                        ob = sd.tile([128, 512], bf16, name="ob")
                        ps = pD.tile([128, 512], f32, name="psd")
                        for gp in range(8):
                            nc.tensor.matmul(
                                ps,
                                ctx_sb[gp][:, qt_i * 128:(qt_i + 1) * 128],
                                wo_sb[gp],
                                start=(gp == 0), stop=(gp == 7),
                            )
                        nc.vector.tensor_copy(ob, ps)
                        nc.sync.dma_start(OUT.ap()[qt_i * 128:(qt_i + 1) * 128, :], ob)

    nc.finalize()
    return nc


def _get_nc():
    global _NC
    if _NC is None:
        _NC = _build()
    return _NC


def _pack_w_into(dst, Wh_bf):
    # dst: contiguous [NP, KT, 128, 128] bf16 view;
    # dst[p,k,i,j] = Wh[2p + j//64, k*128+i, j%64] via one fused strided copy
    dv = dst.reshape(NP, KT, 128, 2, D)
    dv[...] = Wh_bf.reshape(NP, 2, KT, 128, D).transpose(0, 2, 3, 1, 4)


def _global_xs(x):
    import ml_dtypes
    return np.asarray(x, np.float32).reshape(8 * 1024, E).astype(ml_dtypes.bfloat16)


def _global_rest(Wq, bq, Wk, bk, Wv, Wo):
    """Concatenated-over-cores weight/bias arrays (shardable on axis 0)."""
    import ml_dtypes
    bf = ml_dtypes.bfloat16
    Wo = np.asarray(Wo, np.float32)

    # core c = 2b + hh ships its half's pair b of each of q/k/v
    wqkvs = np.empty((NP, 2, 3, KT, 128, 128), bf)
    for wi, W in enumerate((Wq, Wk, Wv)):
        Wb = np.asarray(W, np.float32).astype(bf)
        for hh in range(2):
            _pack_w_into(wqkvs[:, hh, wi], Wb[hh * 8:hh * 8 + 8])
    wqkvs = wqkvs.reshape(8 * 3, KT, 128, 128)

    # Wo transposed, grouped by global head-pair: woT[gp, i, e] = Wo[e, 128*gp + i]
    woT = np.ascontiguousarray(Wo.T).astype(bf).reshape(8, 128, E)
    wos = np.empty((NP, 2, 2, 128, 512), bf)
    for hh in range(2):
        wos[:, hh] = woT[:, :, hh * 512:(hh + 1) * 512].reshape(NP, 2, 128, 512)
    wos = wos.reshape(16, 128, 512)

    bq = np.asarray(bq, np.float32)
    bk = np.asarray(bk, np.float32)
    bqk = np.empty((NP, 2, 2, NP, 128, 1), np.float32)
    for hh in range(2):
        bqk[:, hh, 0] = bq[hh * 8:hh * 8 + 8].reshape(NP, 128, 1)[None]
        bqk[:, hh, 1] = bk[hh * 8:hh * 8 + 8].reshape(NP, 128, 1)[None]
    bqk = bqk.reshape(16, NP, 128, 1)
    return {"wqkvs": wqkvs, "wos": wos, "bqk": bqk}


def build_inputs(x, Wq, bq, Wk, bk, Wv, bv, Wo, bo):
    """Per-core input dicts (list of 8) for the SPMD kernel."""
    xb = _global_xs(x)
    rest = _global_rest(Wq, bq, Wk, bk, Wv, Wo)
    in_maps = []
    for c in range(8):
        in_maps.append({
            "xs": xb[c * 1024:(c + 1) * 1024],
            "wqkvs": rest["wqkvs"][3 * c:3 * c + 3],
            "wos": rest["wos"][2 * c:2 * c + 2],
            "bqk": rest["bqk"][2 * c:2 * c + 2],
        })
    return in_maps


def assemble_output(parts, Wo, bo, bv):
    """parts: [8, S, 512] bf16 -> full [B, S, E] f32 with bias."""
    Wo = np.asarray(Wo, np.float32)
    bo_eff = (np.asarray(bo, np.float32)
              + np.asarray(bv, np.float32).reshape(-1) @ Wo.T)
    out = np.empty((B, S, E), np.float32)
    for c in range(8):
        b, hh = divmod(c, 2)
        cols = slice(hh * 512, (hh + 1) * 512)
        np.add(parts[c], bo_eff[cols], out=out[b, :, cols])
    return out


def _make_runner(nc):
    """Cached jitted shard_map runner over 8 cores (axon / PJRT path).

    Output zero-buffers and the partition-id tensor are created on device
    inside the jitted function, so per call only the real inputs cross the
    host->device link.
    """
    import jax
    import jax.numpy as jnp
    from jax.sharding import Mesh, PartitionSpec, NamedSharding
    try:
        from jax.experimental.shard_map import shard_map
    except ImportError:
        from jax.shard_map import shard_map
    from concourse import bass2jax, mybir
    from concourse.bass2jax import _bass_exec_p, install_neuronx_cc_hook

    install_neuronx_cc_hook()

    partition_name = nc.partition_id_tensor.name if nc.partition_id_tensor else None
    dbg_name = nc.dbg_addr.name if nc.dbg_addr is not None else None

    in_names, out_names, out_avals = [], [], []
    for alloc in nc.m.functions[0].allocations:
        if not isinstance(alloc, mybir.MemoryLocationSet):
            continue
        name = alloc.memorylocations[0].name
        if alloc.kind == "ExternalInput":
            if name not in (partition_name, dbg_name):
                in_names.append(name)
        elif alloc.kind == "ExternalOutput":
            out_names.append(name)
            out_avals.append(jax.core.ShapedArray(
                tuple(alloc.tensor_shape), mybir.dt.np(alloc.dtype)))

    bind_names = list(in_names)
    if dbg_name is not None:
        bind_names.append(dbg_name)
    bind_names.extend(out_names)
    if partition_name is not None:
        bind_names.append(partition_name)

    def _body(*args):
        operands = list(args)
        if partition_name is not None:
            operands.append(bass2jax.partition_id_tensor())
        outs = _bass_exec_p.bind(
            *operands,
            out_avals=tuple(out_avals),
            in_names=tuple(bind_names),
            out_names=tuple(out_names),
            lowering_input_output_aliases=(),
            sim_require_finite=True,
            sim_require_nnan=True,
            nc=nc,
        )
        return tuple(outs)

    devices = jax.devices()[:8]
    mesh = Mesh(np.asarray(devices), ("core",))
    sharding = NamedSharding(mesh, PartitionSpec("core"))
    n_extra = (1 if dbg_name is not None else 0) + len(out_names)
    jitted = jax.jit(shard_map(
        _body, mesh=mesh,
        in_specs=(PartitionSpec("core"),) * (len(in_names) + n_extra),
        out_specs=(PartitionSpec("core"),) * len(out_names),
        check_rep=False,
    ))
    # device-resident zero buffers (output initializers + dbg), reused
    # across calls so they never cross the host->device link again
    extras = []
    if dbg_name is not None:
        extras.append(jax.device_put(np.zeros((8, 2), np.uint32), sharding))
    for a in out_avals:
        extras.append(jax.device_put(
            np.zeros((8 * a.shape[0], *a.shape[1:]), a.dtype), sharding))
    return jitted, in_names, out_names, out_avals, sharding, extras


def _get_runner():
    global _RUNNER
    if _RUNNER is None:
        _RUNNER = _make_runner(_get_nc())
    return _RUNNER


def kernel(x, Wq, bq, Wk, bk, Wv, bv, Wo, bo):
    try:
        import jax
        use_jax = sum(d.platform != "cpu" for d in jax.devices()) >= 8
    except Exception:
        use_jax = False

    if use_jax:
        import jax
        jitted, in_names, out_names, out_avals, sharding, extras = _get_runner()
        # start the big x transfer first; pack weights while it moves
        puts = {"xs": jax.device_put(_global_xs(x), sharding)}
        rest = _global_rest(Wq, bq, Wk, bk, Wv, Wo)
        for nm in ("wqkvs", "wos", "bqk"):
            puts[nm] = jax.device_put(rest[nm], sharding)
        outs = jitted(*(puts[nm] for nm in in_names), *extras)
        i = out_names.index("out")
        parts = np.asarray(outs[i]).reshape(8, *out_avals[i].shape)
    else:
        from concourse.bass_utils import run_bass_kernel_spmd
        in_maps = build_inputs(x, Wq, bq, Wk, bk, Wv, bv, Wo, bo)
        res = run_bass_kernel_spmd(_get_nc(), in_maps, core_ids=list(range(8)))
        parts = np.stack([res.results[c]["out"] for c in range(8)])

    return assemble_output(parts, Wo, bo, bv)
